# revision 19
# baseline (speedup 1.0000x reference)
"""Trainium2 Bass kernel for nn_PointGenerator.

Computes X_world = c2w_c @ [x*d, y*d, d, 1] for 2M points, where
c2w_c = E_c @ n2r @ inv(K_c) is a per-camera (200 cams) affine transform.
The double 4x4 matrix inversion of the reference collapses analytically,
leaving a per-point gather of the camera row (12 coeffs) + a tiny FMA.

Device strategy (8 NeuronCores, pure data parallel over points):
  - Host precomputes the [200, 64]-f32 camera table (row-padded to 256 B).
  - Per core, points live in a [128, F, .] layout (partition p owns F
    consecutive points).  A SWDGE `dma_gather` (4 parallel queues) pulls
    each point's 256 B camera row from HBM into SBUF, landing
    points-on-partitions.  DVE does the 12-term multiply + reduce.
  - Output [128, F, 4] f32 DMAs back contiguously per partition.
"""
import sys

sys.path.insert(0, "/opt/trn_rl_repo")

import numpy as np

N_CORES = 8
F = 1954                 # points per partition per core
NC_PTS = 128 * F         # 250112 points per core (padded)
N_TOTAL = 2_000_000
B_TILE = 64              # gather-tile slots per partition
E = 64                   # f32 per table row (256 B)

_CACHED = {}


def _tile_widths():
    widths = []
    rem = F
    while rem > 0:
        w = min(B_TILE, rem)
        widths.append(w)
        rem -= w
    return widths


def _build_nc():
    from concourse import bacc, tile, mybir

    nc = bacc.Bacc("TRN2", target_bir_lowering=False, debug=False,
                   num_swdge_queues=4, detect_race_conditions=False)
    f32 = mybir.dt.float32
    tab_d = nc.dram_tensor("table", [200, E], f32, kind="ExternalInput")
    pts_d = nc.dram_tensor("pts", [128, F * 3], mybir.dt.int32, kind="ExternalInput")
    dep_d = nc.dram_tensor("dep", [128, F], f32, kind="ExternalInput")
    idx_d = nc.dram_tensor("idx", [128, F * 8], mybir.dt.int16, kind="ExternalInput")
    out_d = nc.dram_tensor("out", [128, F * 4], f32, kind="ExternalOutput")

    qsems = [nc.semaphore(f"qsem{q}").__enter__() for q in range(4)]
    GB = 4                                    # g slot rotation depth
    with tile.TileContext(nc) as tc:
        with tc.tile_pool(name="p", bufs=3) as pool, \
             tc.tile_pool(name="gp", bufs=1) as gpool:
            off = 0
            t = 0
            for B in _tile_widths():
                g = gpool.tile([128, B_TILE, E], f32, name=f"g{t}", tag=f"g{t % GB}")
                # unique per tile: no reuse, so the idx slot can never be
                # overwritten while an in-flight gather still needs it
                it = gpool.tile([128, 8 * B_TILE], mybir.dt.int16, name=f"it{t}", tag=f"it{t}")
                pt = pool.tile([128, B_TILE, 3], mybir.dt.int32, name=f"pt{t}", tag="pt")
                dt_ = pool.tile([128, B_TILE], f32, name=f"dt{t}", tag="dt")
                xf = pool.tile([128, B_TILE], f32, name=f"xf{t}", tag="xf")
                yf = pool.tile([128, B_TILE], f32, name=f"yf{t}", tag="yf")
                v = pool.tile([128, B_TILE, 4], f32, name=f"v{t}", tag="v")
                pr = pool.tile([128, B_TILE, 12], f32, name=f"pr{t}", tag="pr")
                o = pool.tile([128, B_TILE, 4], f32, name=f"o{t}", tag="o")

                nc.sync.dma_start(out=it[:, : 8 * B],
                                  in_=idx_d[:, 8 * off: 8 * (off + B)])
                nc.gpsimd.dma_gather(
                    g[:, :B, :], tab_d[:], it[:, : 8 * B],
                    num_idxs=128 * B, num_idxs_reg=128 * B, elem_size=E,
                    single_packet=False, queue_num=t % 4,
                ).then_inc(qsems[t % 4], 16)
                nc.sync.dma_start(out=pt[:, :B, :], in_=pts_d[:, 3 * off: 3 * (off + B)])
                nc.sync.dma_start(out=dt_[:, :B], in_=dep_d[:, off: off + B])

                # V = [x*d, y*d, d, 1]
                nc.vector.tensor_copy(xf[:, :B], pt[:, :B, 2])
                nc.vector.tensor_copy(yf[:, :B], pt[:, :B, 1])
                nc.vector.tensor_tensor(v[:, :B, 0], xf[:, :B], dt_[:, :B],
                                        mybir.AluOpType.mult)
                nc.vector.tensor_tensor(v[:, :B, 1], yf[:, :B], dt_[:, :B],
                                        mybir.AluOpType.mult)
                nc.vector.tensor_copy(v[:, :B, 2], dt_[:, :B])
                nc.vector.memset(v[:, :B, 3], 1.0)

                # P[p,b,4i+l] = G[p,b,4i+l] * V[p,b,l]; the attached wait
                # gates on the gather's DMA completion (not just desc-gen)
                vrep = v[:, :B].unsqueeze(2).broadcast_to([128, B, 3, 4])
                tt = nc.vector.tensor_tensor(pr[:, :B, :], g[:, :B, :12], vrep,
                                             mybir.AluOpType.mult)
                tt._wait_ge(qsems[t % 4], 16 * (t // 4 + 1))
                # out[p,b,i] = sum_l P[p,b,4i+l]; the reduce depends on pr,
                # so its tsem inc also publishes "TT(t) done, g slot free".
                pr4 = pr[:, :B, :].rearrange("p b (i l) -> p (b i) l", l=4)
                nc.vector.tensor_reduce(
                    o[:, :B, :3], pr4,
                    axis=mybir.AxisListType.X, op=mybir.AluOpType.add)
                nc.vector.memset(o[:, :B, 3], 1.0)

                nc.sync.dma_start(out=out_d[:, 4 * off: 4 * (off + B)],
                                  in_=o[:, :B, :])
                off += B
                t += 1
    nc.compile()
    return nc


def _camera_table(camera_to_worlds, intrinsics):
    c2w = np.asarray(camera_to_worlds, np.float64)   # [C,3,4]
    K = np.asarray(intrinsics, np.float64)           # [C,3,3]
    C = c2w.shape[0]
    E_ = np.zeros((C, 4, 4))
    E_[:, :3, :] = c2w
    E_[:, 3, 3] = 1.0
    n2r = np.diag([1.0, -1.0, -1.0, 1.0])
    K_ = np.zeros((C, 4, 4))
    K_[:, :3, :3] = K
    K_[:, 3, 3] = 1.0
    A = E_ @ n2r[None] @ np.linalg.inv(K_)           # [C,4,4]
    tab = np.zeros((200, E), np.float32)
    tab[:C, :12] = A[:, :3, :].reshape(C, 12).astype(np.float32)
    return tab


def _wrap_idx(ct):
    """ct: [128, F] int16 cameras -> wrapped idx [128, 8F] for dma_gather."""
    blocks = []
    off = 0
    for B in _tile_widths():
        blk = ct[:, off: off + B].reshape(8, 16, B)      # [r, q, j]
        blocks.append(blk.transpose(1, 2, 0).reshape(16, 8 * B))  # slot j*8+r
        off += B
    idx16 = np.concatenate(blocks, axis=1)               # [16, 8F]
    return np.ascontiguousarray(np.broadcast_to(idx16[None], (8, 16, 8 * F))
                                .reshape(128, 8 * F))


def kernel(point_indices, depth, image_coords, camera_to_worlds, intrinsics,
           _trace=False):
    from concourse.bass_utils import run_bass_kernel_spmd

    pi = np.asarray(point_indices)
    if pi.dtype != np.int32:
        pi = pi.astype(np.int32)
    dep = np.ascontiguousarray(np.asarray(depth, np.float32)[:, 0])
    n = pi.shape[0]

    table = _camera_table(camera_to_worlds, intrinsics)

    if "nc" not in _CACHED:
        _CACHED["nc"] = _build_nc()
    nc = _CACHED["nc"]

    in_maps = []
    for k in range(N_CORES):
        lo = k * NC_PTS
        hi = min(lo + NC_PTS, n)
        npts = hi - lo
        if npts == NC_PTS:
            pik = pi[lo:hi]
            depk = dep[lo:hi]
        else:
            pik = np.zeros((NC_PTS, 3), np.int32)
            pik[:npts] = pi[lo:hi]
            depk = np.zeros(NC_PTS, np.float32)
            depk[:npts] = dep[lo:hi]
        ct = np.ascontiguousarray(pik[:, 0].reshape(128, F).astype(np.int16))
        in_maps.append({
            "table": table,
            "pts": pik.reshape(128, F * 3),
            "dep": depk.reshape(128, F),
            "idx": _wrap_idx(ct),
        })

    res = run_bass_kernel_spmd(nc, in_maps, list(range(N_CORES)), trace=_trace)
    _CACHED["last_exec_ns"] = res.exec_time_ns

    out = np.empty((N_CORES * NC_PTS, 4), np.float32)
    for k in range(N_CORES):
        out[k * NC_PTS:(k + 1) * NC_PTS] = res.results[k]["out"].reshape(NC_PTS, 4)
    return out[:n]


# revision 20
# speedup vs baseline: 1.2978x; 1.2978x over previous
"""Trainium2 Bass kernel for nn_PointGenerator.

Computes X_world = c2w_c @ [x*d, y*d, d, 1] for 2M points, where
c2w_c = E_c @ n2r @ inv(K_c) is a per-camera (200 cams) affine transform.
The double 4x4 matrix inversion of the reference collapses analytically,
leaving a per-point gather of the camera row (12 coeffs) + a tiny FMA.

Device strategy (8 NeuronCores, pure data parallel over points):
  - Host precomputes the [200, 64]-f32 camera table (row-padded to 256 B).
  - Per core, points live in a [128, F, .] layout (partition p owns F
    consecutive points).  A SWDGE `dma_gather` (4 parallel queues) pulls
    each point's 256 B camera row from HBM into SBUF, landing
    points-on-partitions.  DVE does the 12-term multiply + reduce.
  - Output [128, F, 4] f32 DMAs back contiguously per partition.
"""
import sys

sys.path.insert(0, "/opt/trn_rl_repo")

import numpy as np

N_CORES = 8
F = 1954                 # points per partition per core
NC_PTS = 128 * F         # 250112 points per core (padded)
N_TOTAL = 2_000_000
B_TILE = 64              # gather-tile slots per partition
E = 64                   # f32 per table row (256 B)

_CACHED = {}


def _tile_widths():
    widths = []
    rem = F
    while rem > 0:
        w = min(B_TILE, rem)
        widths.append(w)
        rem -= w
    return widths


def _build_nc():
    from concourse import bacc, tile, mybir

    nc = bacc.Bacc("TRN2", target_bir_lowering=False, debug=False,
                   num_swdge_queues=4, detect_race_conditions=False)
    f32 = mybir.dt.float32
    tab_d = nc.dram_tensor("table", [200, E], f32, kind="ExternalInput")
    pts_d = nc.dram_tensor("pts", [128, F * 3], mybir.dt.int32, kind="ExternalInput")
    dep_d = nc.dram_tensor("dep", [128, F], f32, kind="ExternalInput")
    idx_d = nc.dram_tensor("idx", [128, F * 8], mybir.dt.int16, kind="ExternalInput")
    out_d = nc.dram_tensor("out", [128, F * 4], f32, kind="ExternalOutput")

    qsems = [nc.semaphore(f"qsem{q}").__enter__() for q in range(4)]
    GB = 8                                    # g slot rotation depth
    with tile.TileContext(nc) as tc:
        with tc.tile_pool(name="p", bufs=3) as pool, \
             tc.tile_pool(name="gp", bufs=1) as gpool:
            off = 0
            t = 0
            for B in _tile_widths():
                g = gpool.tile([128, B_TILE, E], f32, name=f"g{t}", tag=f"g{t % GB}")
                # unique per tile: no reuse, so the idx slot can never be
                # overwritten while an in-flight gather still needs it
                it = gpool.tile([128, 8 * B_TILE], mybir.dt.int16, name=f"it{t}", tag=f"it{t}")
                pt = pool.tile([128, B_TILE, 3], mybir.dt.int32, name=f"pt{t}", tag="pt")
                dt_ = pool.tile([128, B_TILE], f32, name=f"dt{t}", tag="dt")
                xf = pool.tile([128, B_TILE], f32, name=f"xf{t}", tag="xf")
                yf = pool.tile([128, B_TILE], f32, name=f"yf{t}", tag="yf")
                v = pool.tile([128, B_TILE, 4], f32, name=f"v{t}", tag="v")
                pr = pool.tile([128, B_TILE, 12], f32, name=f"pr{t}", tag="pr")
                o = pool.tile([128, B_TILE, 4], f32, name=f"o{t}", tag="o")

                nc.sync.dma_start(out=it[:, : 8 * B],
                                  in_=idx_d[:, 8 * off: 8 * (off + B)])
                nc.gpsimd.dma_gather(
                    g[:, :B, :], tab_d[:], it[:, : 8 * B],
                    num_idxs=128 * B, num_idxs_reg=128 * B, elem_size=E,
                    single_packet=False, queue_num=t % 4,
                ).then_inc(qsems[t % 4], 16)
                nc.sync.dma_start(out=pt[:, :B, :], in_=pts_d[:, 3 * off: 3 * (off + B)])
                nc.sync.dma_start(out=dt_[:, :B], in_=dep_d[:, off: off + B])

                # V = [x*d, y*d, d, 1]
                nc.vector.tensor_copy(xf[:, :B], pt[:, :B, 2])
                nc.vector.tensor_copy(yf[:, :B], pt[:, :B, 1])
                nc.vector.tensor_tensor(v[:, :B, 0], xf[:, :B], dt_[:, :B],
                                        mybir.AluOpType.mult)
                nc.vector.tensor_tensor(v[:, :B, 1], yf[:, :B], dt_[:, :B],
                                        mybir.AluOpType.mult)
                nc.vector.tensor_copy(v[:, :B, 2], dt_[:, :B])
                nc.vector.memset(v[:, :B, 3], 1.0)

                # P[p,b,4i+l] = G[p,b,4i+l] * V[p,b,l]; the attached wait
                # gates on the gather's DMA completion (not just desc-gen)
                vrep = v[:, :B].unsqueeze(2).broadcast_to([128, B, 3, 4])
                tt = nc.vector.tensor_tensor(pr[:, :B, :], g[:, :B, :12], vrep,
                                             mybir.AluOpType.mult)
                tt._wait_ge(qsems[t % 4], 16 * (t // 4 + 1))
                # out[p,b,i] = sum_l P[p,b,4i+l]; the reduce depends on pr,
                # so its tsem inc also publishes "TT(t) done, g slot free".
                pr4 = pr[:, :B, :].rearrange("p b (i l) -> p (b i) l", l=4)
                nc.vector.tensor_reduce(
                    o[:, :B, :3], pr4,
                    axis=mybir.AxisListType.X, op=mybir.AluOpType.add)
                nc.vector.memset(o[:, :B, 3], 1.0)

                nc.sync.dma_start(out=out_d[:, 4 * off: 4 * (off + B)],
                                  in_=o[:, :B, :])
                off += B
                t += 1
    nc.compile()
    return nc


def _camera_table(camera_to_worlds, intrinsics):
    c2w = np.asarray(camera_to_worlds, np.float64)   # [C,3,4]
    K = np.asarray(intrinsics, np.float64)           # [C,3,3]
    C = c2w.shape[0]
    E_ = np.zeros((C, 4, 4))
    E_[:, :3, :] = c2w
    E_[:, 3, 3] = 1.0
    n2r = np.diag([1.0, -1.0, -1.0, 1.0])
    K_ = np.zeros((C, 4, 4))
    K_[:, :3, :3] = K
    K_[:, 3, 3] = 1.0
    A = E_ @ n2r[None] @ np.linalg.inv(K_)           # [C,4,4]
    tab = np.zeros((200, E), np.float32)
    tab[:C, :12] = A[:, :3, :].reshape(C, 12).astype(np.float32)
    return tab


def _wrap_idx(ct):
    """ct: [128, F] int16 cameras -> wrapped idx [128, 8F] for dma_gather."""
    blocks = []
    off = 0
    for B in _tile_widths():
        blk = ct[:, off: off + B].reshape(8, 16, B)      # [r, q, j]
        blocks.append(blk.transpose(1, 2, 0).reshape(16, 8 * B))  # slot j*8+r
        off += B
    idx16 = np.concatenate(blocks, axis=1)               # [16, 8F]
    return np.ascontiguousarray(np.broadcast_to(idx16[None], (8, 16, 8 * F))
                                .reshape(128, 8 * F))


def kernel(point_indices, depth, image_coords, camera_to_worlds, intrinsics,
           _trace=False):
    from concourse.bass_utils import run_bass_kernel_spmd

    pi = np.asarray(point_indices)
    if pi.dtype != np.int32:
        pi = pi.astype(np.int32)
    dep = np.ascontiguousarray(np.asarray(depth, np.float32)[:, 0])
    n = pi.shape[0]

    table = _camera_table(camera_to_worlds, intrinsics)

    if "nc" not in _CACHED:
        _CACHED["nc"] = _build_nc()
    nc = _CACHED["nc"]

    in_maps = []
    for k in range(N_CORES):
        lo = k * NC_PTS
        hi = min(lo + NC_PTS, n)
        npts = hi - lo
        if npts == NC_PTS:
            pik = pi[lo:hi]
            depk = dep[lo:hi]
        else:
            pik = np.zeros((NC_PTS, 3), np.int32)
            pik[:npts] = pi[lo:hi]
            depk = np.zeros(NC_PTS, np.float32)
            depk[:npts] = dep[lo:hi]
        ct = np.ascontiguousarray(pik[:, 0].reshape(128, F).astype(np.int16))
        in_maps.append({
            "table": table,
            "pts": pik.reshape(128, F * 3),
            "dep": depk.reshape(128, F),
            "idx": _wrap_idx(ct),
        })

    res = run_bass_kernel_spmd(nc, in_maps, list(range(N_CORES)), trace=_trace)
    _CACHED["last_exec_ns"] = res.exec_time_ns

    out = np.empty((N_CORES * NC_PTS, 4), np.float32)
    for k in range(N_CORES):
        out[k * NC_PTS:(k + 1) * NC_PTS] = res.results[k]["out"].reshape(NC_PTS, 4)
    return out[:n]
